# revision 33
# baseline (speedup 1.0000x reference)
"""AdaLoRAWithBase distributed Trainium2 kernel (8 NeuronCores).

Strategy (self-contained; shapes hardcoded):
  B=128, T=32, D=1024, ADA=1024, INTER=1024, RANK=8, 8 cores.

  Hypernetwork (ada_emb -> per-sample LoRA factors):
    - LayerNorm(ada_emb) + h = gelu(ae @ w1 + b1) replicated on every core
      (w1 is only 2MB in bf16; replicating beats an AllGather's latency).
    - xw = h @ w2 + b2: each core computes a 2048-col slice of xw for ALL
      128 samples, with w2's columns PRE-PERMUTED on the host so that an
      AllToAll over the batch dim delivers x_a^T / x_b^T in the exact
      [(rank, sample), d] SBUF layout the apply phase needs (32 contiguous
      partitions per source, contiguous 2KB DMA rows, no on-device shuffle).
    - The AllToAll is split into two rank-parity halves so the second
      transfer and the first half's consumer work overlap.
  Apply phase is batch-sharded (16 samples/core):
    out[b] = x[b] @ (base + I + x_a[b] @ x_b[b]^T)
    - T1: X_shard @ (base+I)  (the +I folds in the residual, host-side)
    - T0: P_cross = x_a_batched^T @ X^T with a block-diag mask (computes
      all 16 samples' x@x_a in one 8-matmul chain; mask kills cross terms)
    - T2: one matmul per output tile adds the masked LoRA delta.
  Matmul operands are bf16 (weights/x converted on the host -> halves HBM
  traffic); accumulation is f32 in PSUM; output is f32.
"""

import sys

sys.path.insert(0, "/opt/trn_rl_repo")

import ml_dtypes
import numpy as np

import concourse.bass as bass
import concourse.mybir as mybir
import concourse.tile as tile
from concourse import bacc
from concourse.bass_utils import run_bass_kernel_spmd
from concourse.masks import make_identity

NCORES = 8
B, T, D = 128, 32, 1024
ADA, INTER, RANK = 1024, 1024, 8
BS = B // NCORES            # 16 samples per core
BT = BS * T                 # 512 x-rows per core
CS = 2 * D * RANK // NCORES  # 2048 permuted w2 cols per core
LN_EPS = 1e-5

F32 = mybir.dt.float32
BF16 = mybir.dt.bfloat16
NPBF = ml_dtypes.bfloat16


def build_w2_perm():
    """perm[k*CS + rr*D + d] = original w2 column for (source k, r=2*(k%4)+rr, d).

    Sources 0-3 carry the x_a half, 4-7 the x_b half; each source carries two
    rank indices for all d, d contiguous within each rank block.
    """
    perm = np.empty(2 * D * RANK, dtype=np.int64)
    d = np.arange(D)
    for k in range(NCORES):
        half_off = 0 if k < 4 else D * RANK
        for rr in range(2):
            r = 2 * (k % 4) + rr
            perm[k * CS + rr * D + d] = half_off + d * RANK + r
    return perm


def build_mask():
    """mask[(rr,s,b), (b',t)] = 1.0 iff b == b' (kills P_cross off-diag blocks).

    Row ordering matches the split-A2A delivery: row = rr*64 + s*16 + b,
    carrying rank r = 2s + rr. T0/T2 contract over rows, so any consistent
    ordering of (rank, sample) rows works as long as mask/xaT/xbT agree.
    """
    m = np.zeros((BS * RANK, BS * T), dtype=np.float32)
    for row in range(BS * RANK):
        b = row % BS
        m[row, b * T:(b + 1) * T] = 1.0
    return m


def build_graph(act_gelu=True):
    nc = bacc.Bacc(None, target_bir_lowering=False, debug=False,
                   num_devices=NCORES)

    # -------- DRAM parameters (per-core values supplied via in_maps) --------
    x_d = nc.dram_tensor("x", [BT, D], BF16, kind="ExternalInput")
    ada_d = nc.dram_tensor("ada", [B, ADA], F32, kind="ExternalInput")
    w1_d = nc.dram_tensor("w1t", [128, (ADA // 128) * INTER], BF16,
                          kind="ExternalInput")
    b1_d = nc.dram_tensor("b1t", [128, INTER // 128], F32, kind="ExternalInput")
    w2_d = nc.dram_tensor("w2s", [(CS // 512) * 128, (INTER // 128) * 512], BF16,
                          kind="ExternalInput")
    b2_d = nc.dram_tensor("b2s", [1, CS], F32, kind="ExternalInput")
    base_d = nc.dram_tensor("base", [128, (D // 128) * D], BF16,
                            kind="ExternalInput")
    mask_d = nc.dram_tensor("mask", [BS * RANK, BS * T], F32,
                            kind="ExternalInput")
    out_d = nc.dram_tensor("out", [BT, D], F32, kind="ExternalOutput")

    # -------- internal DRAM bounce buffers for collectives --------
    xw_cin = [nc.dram_tensor(f"xw_cin{h}", [B, CS // 2], BF16) for h in range(2)]
    xw_cout = [nc.dram_tensor(f"xw_cout{h}", [B, CS // 2], BF16) for h in range(2)]

    rg = [list(range(NCORES))]
    KT = D // 128   # 8 contraction tiles

    with tile.TileContext(nc) as tc:
        with (
            tc.tile_pool(name="consts", bufs=1) as consts,
            tc.tile_pool(name="big", bufs=1) as big,
            tc.tile_pool(name="w2p", bufs=4) as w2p,
            tc.tile_pool(name="work", bufs=1) as work,
            tc.tile_pool(name="outp", bufs=1) as outp,
            tc.tile_pool(name="pst", bufs=3, space="PSUM") as pst,
            tc.tile_pool(name="psmm", bufs=3, space="PSUM") as psmm,
            tc.tile_pool(name="psr", bufs=2, space="PSUM") as psr,
        ):
            # ---- front-loaded DMAs; sync queue = hypernet critical path ----
            ae_t = work.tile([B, ADA], F32)
            nc.sync.dma_start(out=ae_t[:], in_=ada_d[:])
            w1_sb = big.tile([128, KT, INTER], BF16)
            nc.sync.dma_start(out=w1_sb[:], in_=w1_d[:])
            b1t_sb = consts.tile([128, KT], F32)
            nc.sync.dma_start(out=b1t_sb[:], in_=b1_d[:])
            w2n_tiles = []
            for n in range(CS // 512):
                w2n = w2p.tile([128, KT, 512], BF16, tag="w2t")
                nc.sync.dma_start(out=w2n[:],
                                  in_=w2_d[n * 128:(n + 1) * 128, :])
                w2n_tiles.append(w2n)

            # ---- constants (ACT HWDGE queue; transposes follow on it) ----
            ident_f = consts.tile([128, 128], F32)
            make_identity(nc, ident_f[:])
            ident_b = consts.tile([128, 128], BF16)
            nc.vector.tensor_copy(ident_b[:], ident_f[:])
            eps_t = consts.tile([128, 1], F32)
            nc.vector.memset(eps_t[:], LN_EPS)
            zero_t = consts.tile([128, 1], F32)
            nc.vector.memset(zero_t[:], 0.0)
            b2_b = consts.tile([128, CS], F32)
            nc.scalar.dma_start(out=b2_b[:], in_=b2_d[:].to_broadcast((128, CS)))
            mask_sb = consts.tile([BS * RANK, BS * T], F32)
            nc.gpsimd.dma_start(out=mask_sb[:], in_=mask_d[:])


            x_tiles = []
            for m in range(BT // 128):
                xt = w2p.tile([128, D], BF16, tag="xt")
                nc.gpsimd.dma_start(out=xt[:], in_=x_d[m * 128:(m + 1) * 128, :])
                x_tiles.append(xt)
            base_sb = big.tile([128, KT, D], BF16)
            nc.gpsimd.dma_start(out=base_sb[:], in_=base_d[:])
            # warm the ACT Gelu table while DMAs stream
            gelu_warm = consts.tile([1, 8], F32)
            nc.vector.memset(gelu_warm[:], 0.0)
            nc.scalar.activation(out=gelu_warm[:], in_=gelu_warm[:],
                                 func=mybir.ActivationFunctionType.Gelu,
                                 bias=zero_t[:1], scale=1.0)

            # ---- LayerNorm (f32) ----
            n_sub = max(1, ADA // nc.vector.BN_STATS_FMAX)
            stats = work.tile([B, n_sub, nc.vector.BN_STATS_DIM], F32)
            ae_v = ae_t[:].rearrange("p (s f) -> p s f", s=n_sub)
            for s in range(n_sub):
                nc.vector.bn_stats(out=stats[:, s, :], in_=ae_v[:, s, :])
            mv = work.tile([B, nc.vector.BN_AGGR_DIM], F32)
            nc.vector.bn_aggr(out=mv[:], in_=stats[:])
            rstd = work.tile([B, 1], F32)
            nc.scalar.activation(out=rstd[:], in_=mv[:, 1:2],
                                 func=mybir.ActivationFunctionType.Sqrt,
                                 bias=eps_t[:], scale=1.0)
            nc.vector.reciprocal(out=rstd[:], in_=rstd[:])
            aen_b = work.tile([B, ADA], BF16)
            nc.vector.tensor_scalar(out=aen_b[:], in0=ae_t[:],
                                    scalar1=mv[:, 0:1], scalar2=rstd[:],
                                    op0=mybir.AluOpType.subtract,
                                    op1=mybir.AluOpType.mult)

            # ae^T tiles [c_local, ct, b] via PE transposes
            aeT = big.tile([128, KT, B], BF16)
            for ct in range(KT):
                ps = pst.tile([128, 128], BF16, tag="ps")
                nc.tensor.transpose(ps[:], aen_b[:, ct * 128:(ct + 1) * 128],
                                    ident_b[:])
                nc.vector.tensor_copy(aeT[:, ct, :], ps[:])

            # ---- h^T computed directly: hT[i,b] = gelu(w1^T @ ae^T + b1) ----
            hT_sb = big.tile([128, KT, B], BF16)
            for it in range(KT):
                h_ps = psmm.tile([128, B], F32, tag="mm")
                for ct in range(KT):
                    nc.tensor.matmul(h_ps[:],
                                     w1_sb[:, ct, it * 128:(it + 1) * 128],
                                     aeT[:, ct, :],
                                     start=(ct == 0), stop=(ct == KT - 1))
                if act_gelu:
                    nc.scalar.activation(out=hT_sb[:, it, :], in_=h_ps[:],
                                         func=mybir.ActivationFunctionType.Gelu,
                                         bias=b1t_sb[:, it:it + 1], scale=1.0)
                else:
                    nc.vector.tensor_add(out=hT_sb[:, it, :], in0=h_ps[:],
                                         in1=b1t_sb[:, it:it + 1].to_broadcast((128, B)))

            # ---- xw slice = h @ w2s + b2s, two halves -> two AllToAlls ----
            xw_sb = work.tile([B, CS], BF16)
            for h in range(2):
                for nn in range(2):
                    n = h * 2 + nn
                    w2n = w2n_tiles[n]
                    xw_ps = psmm.tile([B, 512], F32, tag="mm")
                    for kt in range(KT):
                        nc.tensor.matmul(xw_ps[:], hT_sb[:, kt, :], w2n[:, kt, :],
                                         start=(kt == 0), stop=(kt == KT - 1))
                    nc.vector.tensor_add(out=xw_sb[:, n * 512:(n + 1) * 512],
                                         in0=xw_ps[:],
                                         in1=b2_b[:, n * 512:(n + 1) * 512])
                nc.sync.dma_start(out=xw_cin[h][:],
                                  in_=xw_sb[:, h * 1024:(h + 1) * 1024])
                nc.gpsimd.collective_compute(
                    "AllToAll", mybir.AluOpType.bypass, replica_groups=rg,
                    ins=[xw_cin[h][:].opt()], outs=[xw_cout[h][:].opt()])

            # ---- x^T via PE transposes (fills the A2A wait) ----
            xT_sb = big.tile([128, KT, BT], BF16)
            for m in range(BT // 128):
                for ct in range(KT):
                    ps = pst.tile([128, 128], BF16, tag="ps")
                    nc.tensor.transpose(ps[:], x_tiles[m][:, ct * 128:(ct + 1) * 128],
                                        ident_b[:])
                    nc.vector.tensor_copy(xT_sb[:, ct, m * 128:(m + 1) * 128], ps[:])

            # ---- T1 = X @ (base+I), parked in SBUF during the A2A ----
            ots = {}
            for m in range(BT // 128):
                for n in range(D // 512):
                    R = psr.tile([128, 512], F32, tag="r")
                    for ct in range(KT):
                        nc.tensor.matmul(R[:], xT_sb[:, ct, m * 128:(m + 1) * 128],
                                         base_sb[:, ct, n * 512:(n + 1) * 512],
                                         start=(ct == 0), stop=(ct == KT - 1))
                    ot = outp.tile([128, 512], F32, tag=f"ot{m}{n}")
                    if (m * 2 + n) % 2 == 0:
                        nc.vector.tensor_copy(ot[:], R[:])
                    else:
                        nc.scalar.copy(ot[:], R[:])
                    ots[(m, n)] = ot

            # ---- post-A2A tail (per half: loads, transposes, T0) ----
            # row layout: row = rr*64 + s*16 + b  (rank r = 2s + rr)
            xaT = big.tile([BS * RANK, D], BF16)
            xbT = big.tile([BS * RANK, D], BF16)
            xa_sb = big.tile([128, KT, BS * RANK], BF16)
            pc_ps = psmm.tile([BS * RANK, BS * T], F32, tag="mm")
            pcm_sb = big.tile([BS * RANK, BS * T], BF16)
            for h in range(2):
                for s in range(4):
                    nc.sync.dma_start(
                        out=xaT[h * 64 + s * 16:h * 64 + (s + 1) * 16, :],
                        in_=xw_cout[h][s * BS:(s + 1) * BS, :])
                    nc.scalar.dma_start(
                        out=xbT[h * 64 + s * 16:h * 64 + (s + 1) * 16, :],
                        in_=xw_cout[h][(4 + s) * BS:(5 + s) * BS, :])
                for dt_ in range(KT):
                    ps = pst.tile([128, 64], BF16, tag="ps")
                    nc.tensor.transpose(
                        ps[:], xaT[h * 64:(h + 1) * 64,
                                   dt_ * 128:(dt_ + 1) * 128],
                        ident_b[h * 64:(h + 1) * 64, h * 64:(h + 1) * 64])
                    nc.vector.tensor_copy(
                        xa_sb[:, dt_, h * 64:(h + 1) * 64], ps[:])
                for ct in range(KT):
                    nc.tensor.matmul(pc_ps[h * 64:(h + 1) * 64, :],
                                     xa_sb[:, ct, h * 64:(h + 1) * 64],
                                     xT_sb[:, ct, :],
                                     start=(ct == 0), stop=(ct == KT - 1))
                nc.vector.tensor_mul(out=pcm_sb[h * 64:(h + 1) * 64, :],
                                     in0=pc_ps[h * 64:(h + 1) * 64, :],
                                     in1=mask_sb[h * 64:(h + 1) * 64, :])

            # T2: LoRA delta, added onto the parked T1 tiles, then store
            for m in range(BT // 128):
                for n in range(D // 512):
                    dps = psmm.tile([128, 512], F32, tag="mm")
                    nc.tensor.matmul(dps[:], pcm_sb[:, m * 128:(m + 1) * 128],
                                     xbT[:, n * 512:(n + 1) * 512],
                                     start=True, stop=True)
                    ot = ots[(m, n)]
                    nc.vector.tensor_add(out=ot[:], in0=ot[:], in1=dps[:])
                    nc.sync.dma_start(out=out_d[m * 128:(m + 1) * 128,
                                                n * 512:(n + 1) * 512],
                                      in_=ot[:])

    nc.compile()
    return nc


_GRAPH = None


def _get_graph():
    global _GRAPH
    if _GRAPH is None:
        _GRAPH = build_graph()
    return _GRAPH


def make_in_maps(x, ada_emb, base_layer, w1, b1, w2, b2, ln_g, ln_b):
    x = np.asarray(x, dtype=np.float32)
    ada_emb = np.ascontiguousarray(np.asarray(ada_emb, dtype=np.float32))
    base_layer = np.asarray(base_layer, dtype=np.float32)
    w1 = np.asarray(w1, dtype=np.float32)
    b1 = np.ascontiguousarray(np.asarray(b1, dtype=np.float32).reshape(1, INTER))
    w2 = np.asarray(w2, dtype=np.float32)
    b2 = np.asarray(b2, dtype=np.float32)
    ln_g = np.ascontiguousarray(np.asarray(ln_g, dtype=np.float32).reshape(1, ADA))
    ln_b = np.ascontiguousarray(np.asarray(ln_b, dtype=np.float32).reshape(1, ADA))

    perm = build_w2_perm()
    # fold LayerNorm gain/bias into w1/b1:  (aen*g + b) @ w1 + b1
    w1_f = w1 * ln_g.reshape(ADA, 1)
    b1_f = b1 + ln_b.reshape(1, ADA) @ w1
    # device layouts: w1t [p, (ct i)], b1t [p, kt], w2 per-n-chunk contiguous,
    # base [p, (ct n)]
    w1_t = np.ascontiguousarray(
        w1_f.astype(NPBF).reshape(8, 128, INTER).transpose(1, 0, 2)
        .reshape(128, 8 * INTER))
    b1_t = np.ascontiguousarray(b1_f.reshape(INTER // 128, 128).T)
    w2p_ = w2[:, perm].astype(NPBF)
    b2p_ = np.ascontiguousarray(b2[perm]).reshape(1, 2 * D * RANK)
    base_p = np.ascontiguousarray(
        (base_layer + np.eye(D, dtype=np.float32)).astype(NPBF)
        .reshape(8, 128, D).transpose(1, 0, 2).reshape(128, 8 * D))
    x_b = x.reshape(B, T, D).astype(NPBF)
    mask = build_mask()

    in_maps = []
    for k in range(NCORES):
        w2k = w2p_[:, k * CS:(k + 1) * CS]       # (INTER, CS)
        w2k_t = np.ascontiguousarray(
            w2k.reshape(8, 128, CS // 512, 512).transpose(2, 1, 0, 3)
            .reshape((CS // 512) * 128, 8 * 512))
        in_maps.append({
            "x": np.ascontiguousarray(
                x_b[k * BS:(k + 1) * BS].reshape(BT, D)),
            "ada": ada_emb,
            "w1t": w1_t,
            "b1t": b1_t,
            "w2s": w2k_t,
            "b2s": np.ascontiguousarray(b2p_[:, k * CS:(k + 1) * CS]),
            "base": base_p,
            "mask": mask,
        })
    return in_maps


def kernel(x, ada_emb, base_layer, w1, b1, w2, b2, ln_g, ln_b, _trace=False,
           _trace_cores=None, _tmpdir=None):
    nc = _get_graph()
    in_maps = make_in_maps(x, ada_emb, base_layer, w1, b1, w2, b2, ln_g, ln_b)
    res = None
    for attempt in range(3):
        try:
            res = run_bass_kernel_spmd(nc, in_maps, core_ids=list(range(NCORES)),
                                       trace=_trace, trace_cores=_trace_cores,
                                       tmpdir=_tmpdir)
            break
        except Exception:
            # transient NRT_EXEC_UNIT_UNRECOVERABLE-style failures recover on
            # retry (observed once on a cold device); re-raise on the last try
            if attempt == 2:
                raise
    out = np.concatenate(
        [np.asarray(res.results[i]["out"]).reshape(BS, T, D)
         for i in range(NCORES)], axis=0)
    if _trace:
        kernel.last_exec_time_ns = res.exec_time_ns
        kernel.last_results = res
    return out


# revision 34
# speedup vs baseline: 1.0384x; 1.0384x over previous
"""AdaLoRAWithBase distributed Trainium2 kernel (8 NeuronCores).

Strategy (self-contained; shapes hardcoded):
  B=128, T=32, D=1024, ADA=1024, INTER=1024, RANK=8, 8 cores.

  Hypernetwork (ada_emb -> per-sample LoRA factors):
    - LayerNorm(ada_emb) + h = gelu(ae @ w1 + b1) replicated on every core
      (w1 is only 2MB in bf16; replicating beats an AllGather's latency).
    - xw = h @ w2 + b2: each core computes a 2048-col slice of xw for ALL
      128 samples, with w2's columns PRE-PERMUTED on the host so that an
      AllToAll over the batch dim delivers x_a^T / x_b^T in the exact
      [(rank, sample), d] SBUF layout the apply phase needs (32 contiguous
      partitions per source, contiguous 2KB DMA rows, no on-device shuffle).
    - The AllToAll is split into two rank-parity halves so the second
      transfer and the first half's consumer work overlap.
  Apply phase is batch-sharded (16 samples/core):
    out[b] = x[b] @ (base + I + x_a[b] @ x_b[b]^T)
    - T1: X_shard @ (base+I)  (the +I folds in the residual, host-side)
    - T0: P_cross = x_a_batched^T @ X^T with a block-diag mask (computes
      all 16 samples' x@x_a in one 8-matmul chain; mask kills cross terms)
    - T2: one matmul per output tile adds the masked LoRA delta.
  Matmul operands are bf16 (weights/x converted on the host -> halves HBM
  traffic); accumulation is f32 in PSUM; output is f32.
"""

import sys

sys.path.insert(0, "/opt/trn_rl_repo")

import ml_dtypes
import numpy as np

import concourse.bass as bass
import concourse.mybir as mybir
import concourse.tile as tile
from concourse import bacc
from concourse.bass_utils import run_bass_kernel_spmd
from concourse.masks import make_identity

NCORES = 8
B, T, D = 128, 32, 1024
ADA, INTER, RANK = 1024, 1024, 8
BS = B // NCORES            # 16 samples per core
BT = BS * T                 # 512 x-rows per core
CS = 2 * D * RANK // NCORES  # 2048 permuted w2 cols per core
LN_EPS = 1e-5

F32 = mybir.dt.float32
BF16 = mybir.dt.bfloat16
NPBF = ml_dtypes.bfloat16


def build_w2_perm():
    """perm[k*CS + rr*D + d] = original w2 column for (source k, r=2*(k%4)+rr, d).

    Sources 0-3 carry the x_a half, 4-7 the x_b half; each source carries two
    rank indices for all d, d contiguous within each rank block.
    """
    perm = np.empty(2 * D * RANK, dtype=np.int64)
    d = np.arange(D)
    for k in range(NCORES):
        half_off = 0 if k < 4 else D * RANK
        for rr in range(2):
            r = 2 * (k % 4) + rr
            perm[k * CS + rr * D + d] = half_off + d * RANK + r
    return perm


def build_mask():
    """mask[(rr,s,b), (b',t)] = 1.0 iff b == b' (kills P_cross off-diag blocks).

    Row ordering matches the split-A2A delivery: row = rr*64 + s*16 + b,
    carrying rank r = 2s + rr. T0/T2 contract over rows, so any consistent
    ordering of (rank, sample) rows works as long as mask/xaT/xbT agree.
    """
    m = np.zeros((BS * RANK, BS * T), dtype=np.float32)
    for row in range(BS * RANK):
        b = row % BS
        m[row, b * T:(b + 1) * T] = 1.0
    return m


def build_graph(act_gelu=True):
    nc = bacc.Bacc(None, target_bir_lowering=False, debug=False,
                   num_devices=NCORES)

    # -------- DRAM parameters (per-core values supplied via in_maps) --------
    x_d = nc.dram_tensor("x", [BT, D], BF16, kind="ExternalInput")
    ada_d = nc.dram_tensor("ada", [B, ADA], F32, kind="ExternalInput")
    w1_d = nc.dram_tensor("w1t", [128, (ADA // 128) * INTER], BF16,
                          kind="ExternalInput")
    b1_d = nc.dram_tensor("b1t", [128, INTER // 128], F32, kind="ExternalInput")
    w2_d = nc.dram_tensor("w2s", [(CS // 512) * 128, (INTER // 128) * 512], BF16,
                          kind="ExternalInput")
    b2_d = nc.dram_tensor("b2s", [1, CS], F32, kind="ExternalInput")
    base_d = nc.dram_tensor("base", [128, (D // 128) * D], BF16,
                            kind="ExternalInput")
    mask_d = nc.dram_tensor("mask", [BS * RANK, BS * T], F32,
                            kind="ExternalInput")
    out_d = nc.dram_tensor("out", [BT, D], F32, kind="ExternalOutput")

    # -------- internal DRAM bounce buffers for collectives --------
    xw_cin = [nc.dram_tensor(f"xw_cin{h}", [B, CS // 2], BF16) for h in range(2)]
    xw_cout = [nc.dram_tensor(f"xw_cout{h}", [B, CS // 2], BF16) for h in range(2)]

    rg = [list(range(NCORES))]
    KT = D // 128   # 8 contraction tiles

    with tile.TileContext(nc) as tc:
        with (
            tc.tile_pool(name="consts", bufs=1) as consts,
            tc.tile_pool(name="big", bufs=1) as big,
            tc.tile_pool(name="w2p", bufs=4) as w2p,
            tc.tile_pool(name="work", bufs=1) as work,
            tc.tile_pool(name="outp", bufs=1) as outp,
            tc.tile_pool(name="pst", bufs=4, space="PSUM") as pst,
            tc.tile_pool(name="psmm", bufs=2, space="PSUM") as psmm,
            tc.tile_pool(name="psr", bufs=2, space="PSUM") as psr,
        ):
            # ---- front-loaded DMAs; sync queue = hypernet critical path ----
            ae_t = work.tile([B, ADA], F32)
            nc.sync.dma_start(out=ae_t[:], in_=ada_d[:])
            w1_sb = big.tile([128, KT, INTER], BF16)
            nc.sync.dma_start(out=w1_sb[:], in_=w1_d[:])
            b1t_sb = consts.tile([128, KT], F32)
            nc.sync.dma_start(out=b1t_sb[:], in_=b1_d[:])
            w2n_tiles = []
            for n in range(CS // 512):
                w2n = w2p.tile([128, KT, 512], BF16, tag="w2t")
                nc.sync.dma_start(out=w2n[:],
                                  in_=w2_d[n * 128:(n + 1) * 128, :])
                w2n_tiles.append(w2n)

            # ---- constants (ACT HWDGE queue; transposes follow on it) ----
            ident_f = consts.tile([128, 128], F32)
            make_identity(nc, ident_f[:])
            ident_b = consts.tile([128, 128], BF16)
            nc.vector.tensor_copy(ident_b[:], ident_f[:])
            eps_t = consts.tile([128, 1], F32)
            nc.vector.memset(eps_t[:], LN_EPS)
            zero_t = consts.tile([128, 1], F32)
            nc.vector.memset(zero_t[:], 0.0)
            b2_b = consts.tile([128, CS], F32)
            nc.scalar.dma_start(out=b2_b[:], in_=b2_d[:].to_broadcast((128, CS)))
            mask_sb = consts.tile([BS * RANK, BS * T], F32)
            nc.gpsimd.dma_start(out=mask_sb[:], in_=mask_d[:])


            x_tiles = []
            for m in range(BT // 128):
                xt = w2p.tile([128, D], BF16, tag="xt")
                nc.gpsimd.dma_start(out=xt[:], in_=x_d[m * 128:(m + 1) * 128, :])
                x_tiles.append(xt)
            base_sb = big.tile([128, KT, D], BF16)
            nc.gpsimd.dma_start(out=base_sb[:], in_=base_d[:])
            # warm the ACT Gelu table while DMAs stream
            gelu_warm = consts.tile([1, 8], F32)
            nc.vector.memset(gelu_warm[:], 0.0)
            nc.scalar.activation(out=gelu_warm[:], in_=gelu_warm[:],
                                 func=mybir.ActivationFunctionType.Gelu,
                                 bias=zero_t[:1], scale=1.0)

            # ---- LayerNorm (f32) ----
            n_sub = max(1, ADA // nc.vector.BN_STATS_FMAX)
            stats = work.tile([B, n_sub, nc.vector.BN_STATS_DIM], F32)
            ae_v = ae_t[:].rearrange("p (s f) -> p s f", s=n_sub)
            for s in range(n_sub):
                nc.vector.bn_stats(out=stats[:, s, :], in_=ae_v[:, s, :])
            mv = work.tile([B, nc.vector.BN_AGGR_DIM], F32)
            nc.vector.bn_aggr(out=mv[:], in_=stats[:])
            rstd = work.tile([B, 1], F32)
            nc.scalar.activation(out=rstd[:], in_=mv[:, 1:2],
                                 func=mybir.ActivationFunctionType.Sqrt,
                                 bias=eps_t[:], scale=1.0)
            nc.vector.reciprocal(out=rstd[:], in_=rstd[:])
            aen_b = work.tile([B, ADA], BF16)
            nc.vector.tensor_scalar(out=aen_b[:], in0=ae_t[:],
                                    scalar1=mv[:, 0:1], scalar2=rstd[:],
                                    op0=mybir.AluOpType.subtract,
                                    op1=mybir.AluOpType.mult)

            # ae^T tiles [c_local, ct, b] via PE transposes
            aeT = big.tile([128, KT, B], BF16)
            for ct in range(KT):
                ps = pst.tile([128, 128], BF16, tag="ps")
                nc.tensor.transpose(ps[:], aen_b[:, ct * 128:(ct + 1) * 128],
                                    ident_b[:])
                nc.vector.tensor_copy(aeT[:, ct, :], ps[:])

            # ---- h^T computed directly: hT[i,b] = gelu(w1^T @ ae^T + b1) ----
            hT_sb = big.tile([128, KT, B], BF16)
            for it in range(KT):
                h_ps = psmm.tile([128, B], F32, tag="mm")
                for ct in range(KT):
                    nc.tensor.matmul(h_ps[:],
                                     w1_sb[:, ct, it * 128:(it + 1) * 128],
                                     aeT[:, ct, :],
                                     start=(ct == 0), stop=(ct == KT - 1))
                if act_gelu:
                    nc.scalar.activation(out=hT_sb[:, it, :], in_=h_ps[:],
                                         func=mybir.ActivationFunctionType.Gelu,
                                         bias=b1t_sb[:, it:it + 1], scale=1.0)
                else:
                    nc.vector.tensor_add(out=hT_sb[:, it, :], in0=h_ps[:],
                                         in1=b1t_sb[:, it:it + 1].to_broadcast((128, B)))

            # ---- xw slice = h @ w2s + b2s, two halves -> two AllToAlls ----
            xw_sb = work.tile([B, CS], BF16)
            for h in range(2):
                for nn in range(2):
                    n = h * 2 + nn
                    w2n = w2n_tiles[n]
                    xw_ps = psmm.tile([B, 512], F32, tag="mm")
                    for kt in range(KT):
                        nc.tensor.matmul(xw_ps[:], hT_sb[:, kt, :], w2n[:, kt, :],
                                         start=(kt == 0), stop=(kt == KT - 1))
                    nc.vector.tensor_add(out=xw_sb[:, n * 512:(n + 1) * 512],
                                         in0=xw_ps[:],
                                         in1=b2_b[:, n * 512:(n + 1) * 512])
                nc.sync.dma_start(out=xw_cin[h][:],
                                  in_=xw_sb[:, h * 1024:(h + 1) * 1024])
                nc.gpsimd.collective_compute(
                    "AllToAll", mybir.AluOpType.bypass, replica_groups=rg,
                    ins=[xw_cin[h][:].opt()], outs=[xw_cout[h][:].opt()])

            # ---- x^T via PE transposes (fills the A2A wait) ----
            xT_sb = big.tile([128, KT, BT], BF16)
            for m in range(BT // 128):
                for ct in range(KT):
                    ps = pst.tile([128, 128], BF16, tag="ps")
                    nc.tensor.transpose(ps[:], x_tiles[m][:, ct * 128:(ct + 1) * 128],
                                        ident_b[:])
                    nc.vector.tensor_copy(xT_sb[:, ct, m * 128:(m + 1) * 128], ps[:])

            # ---- T1 = X @ (base+I), parked in SBUF during the A2A ----
            ots = {}
            for m in range(BT // 128):
                for n in range(D // 512):
                    R = psr.tile([128, 512], F32, tag="r")
                    for ct in range(KT):
                        nc.tensor.matmul(R[:], xT_sb[:, ct, m * 128:(m + 1) * 128],
                                         base_sb[:, ct, n * 512:(n + 1) * 512],
                                         start=(ct == 0), stop=(ct == KT - 1))
                    ot = outp.tile([128, 512], F32, tag=f"ot{m}{n}")
                    if (m * 2 + n) % 2 == 0:
                        nc.vector.tensor_copy(ot[:], R[:])
                    else:
                        nc.scalar.copy(ot[:], R[:])
                    ots[(m, n)] = ot

            # ---- post-A2A tail (per half: loads, transposes, T0) ----
            # row layout: row = rr*64 + s*16 + b  (rank r = 2s + rr)
            xaT = big.tile([BS * RANK, D], BF16)
            xbT = big.tile([BS * RANK, D], BF16)
            xa_sb = big.tile([128, KT, BS * RANK], BF16)
            pc_ps = psmm.tile([BS * RANK, BS * T], F32, tag="mm")
            pcm_sb = big.tile([BS * RANK, BS * T], BF16)
            for h in range(2):
                for s in range(4):
                    nc.sync.dma_start(
                        out=xaT[h * 64 + s * 16:h * 64 + (s + 1) * 16, :],
                        in_=xw_cout[h][s * BS:(s + 1) * BS, :])
                    nc.scalar.dma_start(
                        out=xbT[h * 64 + s * 16:h * 64 + (s + 1) * 16, :],
                        in_=xw_cout[h][(4 + s) * BS:(5 + s) * BS, :])
                for dt_ in range(KT):
                    ps = pst.tile([128, 64], BF16, tag="ps")
                    nc.tensor.transpose(
                        ps[:], xaT[h * 64:(h + 1) * 64,
                                   dt_ * 128:(dt_ + 1) * 128],
                        ident_b[h * 64:(h + 1) * 64, h * 64:(h + 1) * 64])
                    nc.vector.tensor_copy(
                        xa_sb[:, dt_, h * 64:(h + 1) * 64], ps[:])
                for ct in range(KT):
                    nc.tensor.matmul(pc_ps[h * 64:(h + 1) * 64, :],
                                     xa_sb[:, ct, h * 64:(h + 1) * 64],
                                     xT_sb[:, ct, :],
                                     start=(ct == 0), stop=(ct == KT - 1))
                nc.vector.tensor_mul(out=pcm_sb[h * 64:(h + 1) * 64, :],
                                     in0=pc_ps[h * 64:(h + 1) * 64, :],
                                     in1=mask_sb[h * 64:(h + 1) * 64, :])

            # T2: LoRA delta, added onto the parked T1 tiles, then store
            for m in range(BT // 128):
                for n in range(D // 512):
                    dps = psmm.tile([128, 512], F32, tag="mm")
                    nc.tensor.matmul(dps[:], pcm_sb[:, m * 128:(m + 1) * 128],
                                     xbT[:, n * 512:(n + 1) * 512],
                                     start=True, stop=True)
                    ot = ots[(m, n)]
                    nc.vector.tensor_add(out=ot[:], in0=ot[:], in1=dps[:])
                    nc.sync.dma_start(out=out_d[m * 128:(m + 1) * 128,
                                                n * 512:(n + 1) * 512],
                                      in_=ot[:])

    nc.compile()
    return nc


_GRAPH = None


def _get_graph():
    global _GRAPH
    if _GRAPH is None:
        _GRAPH = build_graph()
    return _GRAPH


def make_in_maps(x, ada_emb, base_layer, w1, b1, w2, b2, ln_g, ln_b):
    x = np.asarray(x, dtype=np.float32)
    ada_emb = np.ascontiguousarray(np.asarray(ada_emb, dtype=np.float32))
    base_layer = np.asarray(base_layer, dtype=np.float32)
    w1 = np.asarray(w1, dtype=np.float32)
    b1 = np.ascontiguousarray(np.asarray(b1, dtype=np.float32).reshape(1, INTER))
    w2 = np.asarray(w2, dtype=np.float32)
    b2 = np.asarray(b2, dtype=np.float32)
    ln_g = np.ascontiguousarray(np.asarray(ln_g, dtype=np.float32).reshape(1, ADA))
    ln_b = np.ascontiguousarray(np.asarray(ln_b, dtype=np.float32).reshape(1, ADA))

    perm = build_w2_perm()
    # fold LayerNorm gain/bias into w1/b1:  (aen*g + b) @ w1 + b1
    w1_f = w1 * ln_g.reshape(ADA, 1)
    b1_f = b1 + ln_b.reshape(1, ADA) @ w1
    # device layouts: w1t [p, (ct i)], b1t [p, kt], w2 per-n-chunk contiguous,
    # base [p, (ct n)]
    w1_t = np.ascontiguousarray(
        w1_f.astype(NPBF).reshape(8, 128, INTER).transpose(1, 0, 2)
        .reshape(128, 8 * INTER))
    b1_t = np.ascontiguousarray(b1_f.reshape(INTER // 128, 128).T)
    w2p_ = w2[:, perm].astype(NPBF)
    b2p_ = np.ascontiguousarray(b2[perm]).reshape(1, 2 * D * RANK)
    base_p = np.ascontiguousarray(
        (base_layer + np.eye(D, dtype=np.float32)).astype(NPBF)
        .reshape(8, 128, D).transpose(1, 0, 2).reshape(128, 8 * D))
    x_b = x.reshape(B, T, D).astype(NPBF)
    mask = build_mask()

    in_maps = []
    for k in range(NCORES):
        w2k = w2p_[:, k * CS:(k + 1) * CS]       # (INTER, CS)
        w2k_t = np.ascontiguousarray(
            w2k.reshape(8, 128, CS // 512, 512).transpose(2, 1, 0, 3)
            .reshape((CS // 512) * 128, 8 * 512))
        in_maps.append({
            "x": np.ascontiguousarray(
                x_b[k * BS:(k + 1) * BS].reshape(BT, D)),
            "ada": ada_emb,
            "w1t": w1_t,
            "b1t": b1_t,
            "w2s": w2k_t,
            "b2s": np.ascontiguousarray(b2p_[:, k * CS:(k + 1) * CS]),
            "base": base_p,
            "mask": mask,
        })
    return in_maps


def kernel(x, ada_emb, base_layer, w1, b1, w2, b2, ln_g, ln_b, _trace=False,
           _trace_cores=None, _tmpdir=None):
    nc = _get_graph()
    in_maps = make_in_maps(x, ada_emb, base_layer, w1, b1, w2, b2, ln_g, ln_b)
    res = None
    for attempt in range(3):
        try:
            res = run_bass_kernel_spmd(nc, in_maps, core_ids=list(range(NCORES)),
                                       trace=_trace, trace_cores=_trace_cores,
                                       tmpdir=_tmpdir)
            break
        except Exception:
            # transient NRT_EXEC_UNIT_UNRECOVERABLE-style failures recover on
            # retry (observed once on a cold device); re-raise on the last try
            if attempt == 2:
                raise
    out = np.concatenate(
        [np.asarray(res.results[i]["out"]).reshape(BS, T, D)
         for i in range(NCORES)], axis=0)
    if _trace:
        kernel.last_exec_time_ns = res.exec_time_ns
        kernel.last_results = res
    return out
